# revision 3
# baseline (speedup 1.0000x reference)
"""Trainium2 Bass kernel for AdaptiveProjection (dense MoE routing).

Computes: out[b,s,:] = sum_e softmax(x@gate_w.T + gate_b)[b,s,e] * (x[b,s] @ W_e.T)

Sharding: data-parallel over (B*S) across 8 NeuronCores; weights replicated.
Per core: 2048 tokens, full 4x1024x1024 expert weights resident in SBUF (bf16).
"""

import numpy as np
import ml_dtypes

B, S, D, O, E = 4, 4096, 1024, 1024, 4
N_CORES = 8
T = (B * S) // N_CORES  # 2048 tokens per core
KC = D // 128           # 8 contraction chunks of 128
NT = T // 128           # 16 token tiles per core
NH = O // 512           # 2 output halves
BLK = 512               # gate-logit token block

_CACHE = {}


def _build_graph():
    import concourse.mybir as mybir
    from concourse import bacc
    from concourse.bass import ts, ds
    from concourse.tile import TileContext

    f32 = mybir.dt.float32
    bf16 = mybir.dt.bfloat16
    nc = bacc.Bacc(None, target_bir_lowering=False)

    xt_d = nc.declare_dram_parameter("xt", [KC, 128, T], bf16, isOutput=False)
    wt_d = nc.declare_dram_parameter("wt", [KC, 128, E, O], bf16, isOutput=False)
    gwt_d = nc.declare_dram_parameter("gwt", [KC, 128, E], bf16, isOutput=False)
    gb_d = nc.declare_dram_parameter("gb", [E, 1], f32, isOutput=False)
    id_d = nc.declare_dram_parameter("ident", [E, E], f32, isOutput=False)
    out_d = nc.declare_dram_parameter("out", [T, O], f32, isOutput=True)

    with TileContext(nc) as tc:
        with (
            tc.tile_pool(name="persist", bufs=1) as pp,
            tc.tile_pool(name="gate_sm", bufs=4) as gp,
            tc.tile_pool(name="acc", bufs=6) as ap,
        ):
            # --- persistent SBUF tensors ---
            xt_sb = pp.tile([128, KC, T], bf16, tag="xt")
            w_sb = pp.tile([128, KC, E, O], bf16, tag="w")
            gw_sb = pp.tile([128, KC, E], bf16, tag="gw")
            gb_sb = pp.tile([E, 1], f32, tag="gb")
            id_sb = pp.tile([E, E], f32, tag="ident")
            exp_sb = pp.tile([E, T], f32, tag="exprow")
            gates_sb = pp.tile([128, NT * E], f32, tag="gates")

            # --- loads (HWDGE via sync engine) ---
            nc.sync.dma_start(out=gb_sb[:, :], in_=gb_d[:, :])
            nc.sync.dma_start(out=id_sb[:, :], in_=id_d[:, :])
            nc.sync.dma_start(
                out=gw_sb[:, :, :], in_=gwt_d.rearrange("k p e -> p k e")
            )
            for k in range(KC):
                nc.sync.dma_start(out=xt_sb[:, k, :], in_=xt_d[k])
            for k in range(KC):
                nc.sync.dma_start(out=w_sb[:, k, :, :], in_=wt_d[k])

            # --- gate prologue ---
            # logits in row layout [E, tokens]: stationary = gwT chunk [128, E]
            with tc.tile_pool(name="psum_g", bufs=2, space="PSUM") as pgp:
                for b in range(T // BLK):
                    glog = pgp.tile([E, BLK], f32, tag="glog")
                    for k in range(KC):
                        nc.tensor.matmul(
                            glog[:, :],
                            gw_sb[:, k, :],
                            xt_sb[:, k, ts(b, BLK)],
                            start=(k == 0),
                            stop=(k == KC - 1),
                        )
                    # exp(logits + gate_b) on ACT; bias is per-partition [E,1]
                    nc.scalar.activation(
                        exp_sb[:, ts(b, BLK)],
                        glog[:, :],
                        mybir.ActivationFunctionType.Exp,
                        bias=gb_sb[:, 0:1],
                        scale=1.0,
                    )

            # transpose exp rows -> [128, E] per token tile; normalize
            with tc.tile_pool(name="psum_t", bufs=1, space="PSUM") as ptp:
                expT = ptp.tile([128, NT * E], f32, tag="expT")
                for t in range(NT):
                    nc.tensor.transpose(
                        expT[:, ts(t, E)],
                        exp_sb[:, ts(t, 128)],
                        id_sb[:, :],
                    )
                for t in range(NT):
                    denom = gp.tile([128, 1], f32, tag="denom")
                    recip = gp.tile([128, 1], f32, tag="recip")
                    nc.vector.reduce_sum(
                        denom[:, :], expT[:, ts(t, E)], axis=mybir.AxisListType.X
                    )
                    nc.vector.reciprocal(recip[:, :], denom[:, :])
                    nc.vector.tensor_scalar_mul(
                        gates_sb[:, ts(t, E)], expT[:, ts(t, E)], recip[:, 0:1]
                    )

            # --- main loop: expert matmuls + gated combine ---
            with tc.tile_pool(name="psum_e", bufs=8, space="PSUM") as pep:
                for t in range(NT):
                    for h in range(NH):
                        psums = [
                            pep.tile([128, 512], f32, tag="ep", name=f"ep{t}_{h}_{e}")
                            for e in range(E)
                        ]
                        for k in range(KC):
                            lhs = xt_sb[:, k, ts(t, 128)]
                            for e in range(E):
                                nc.tensor.matmul(
                                    psums[e][:, :],
                                    lhs,
                                    w_sb[:, k, e, ds(512 * h, 512)],
                                    start=(k == 0),
                                    stop=(k == KC - 1),
                                )
                        acc = ap.tile([128, 512], f32, tag="acc")
                        # acc = psum0 * g0   (ACT, per-partition scale)
                        nc.scalar.activation(
                            acc[:, :],
                            psums[0][:, :],
                            mybir.ActivationFunctionType.Copy,
                            bias=0.0,
                            scale=gates_sb[:, t * E : t * E + 1],
                        )
                        # acc = psum_e * g_e + acc  (DVE fused)
                        for e in range(1, E):
                            nc.vector.scalar_tensor_tensor(
                                acc[:, :],
                                psums[e][:, :],
                                gates_sb[:, t * E + e : t * E + e + 1],
                                acc[:, :],
                                op0=mybir.AluOpType.mult,
                                op1=mybir.AluOpType.add,
                            )
                        nc.scalar.dma_start(
                            out=out_d[ts(t, 128), ds(512 * h, 512)], in_=acc[:, :]
                        )
    nc.compile()
    return nc


def _prep_inputs(x, W_experts, gate_w, gate_b):
    bf16 = ml_dtypes.bfloat16
    x_flat = np.asarray(x, dtype=np.float32).reshape(B * S, D)
    # weights (replicated): wt[k,p,e,o] = W[e,o,128k+p]
    wt = (
        np.ascontiguousarray(
            np.asarray(W_experts, dtype=np.float32).transpose(2, 0, 1)
        )
        .reshape(KC, 128, E, O)
        .astype(bf16)
    )
    gwt = (
        np.ascontiguousarray(np.asarray(gate_w, dtype=np.float32).T)
        .reshape(KC, 128, E)
        .astype(bf16)
    )
    gb = np.asarray(gate_b, dtype=np.float32).reshape(E, 1)
    ident = np.eye(E, dtype=np.float32)

    in_maps = []
    for i in range(N_CORES):
        shard = x_flat[i * T : (i + 1) * T]  # [T, D]
        xt = (
            np.ascontiguousarray(shard.T).reshape(KC, 128, T).astype(bf16)
        )
        in_maps.append(
            {"xt": xt, "wt": wt, "gwt": gwt, "gb": gb, "ident": ident}
        )
    return in_maps


def _run(inputs, trace=False):
    from concourse.bass_utils import run_bass_kernel_spmd

    if "nc" not in _CACHE:
        _CACHE["nc"] = _build_graph()
    nc = _CACHE["nc"]
    in_maps = _prep_inputs(**inputs)
    res = run_bass_kernel_spmd(
        nc, in_maps, core_ids=list(range(N_CORES)), trace=trace
    )
    shards = [np.asarray(res.results[i]["out"]) for i in range(N_CORES)]
    out = np.concatenate(shards, axis=0).reshape(B, S, O).astype(np.float32)
    return out, res


def kernel(x, W_experts, gate_w, gate_b):
    out, _ = _run(
        {"x": x, "W_experts": W_experts, "gate_w": gate_w, "gate_b": gate_b}
    )
    return out
